# revision 1
# baseline (speedup 1.0000x reference)
"""Single-head attention on 8 TRN2 NeuronCores, data-parallel over batch.

Per core (batch b):
  qT/kT/vT [64,2048] = W @ x.T   (x.T via PE transposes of natural-layout loads)
  scores[s,t] tiles -> exp(scale*x) with fused row-sum accum -> normalize (DVE)
    -> attention_score output rows
  scoresT[t,s] tiles -> exp -> PV matmul (V natural via vT transpose) -> attT
    -> transpose + per-row 1/sum -> attention output

All matmuls run as float32r (full-rate fp32 on the PE at N>=512).
"""

import numpy as np

B, S, E, D = 8, 2048, 1024, 64
N_CORES = 8
NSC = S // 128   # 16 s-chunks
NEC = E // 128   # 8  e-chunks
NSB = S // 512   # 4  s-blocks
SCALE = 0.125    # 1 / sqrt(D)

MAX_SYNC_WAITS = 1

_CACHE = {}


def _spread_sync_waits(nc, mybir, max_waits=MAX_SYNC_WAITS):
    """This walrus rejects instructions with more than `max_waits` sync-waits
    ("Too many sync wait commands"). Hoist excess waits onto same-engine NoOps
    placed immediately before the instruction."""
    counter = 0
    for f in nc.m.functions:
        for blk in f.blocks:
            out = []
            for inst in blk.instructions:
                si = getattr(inst, "sync_info", None)
                waits = list(si.on_wait) if si is not None and si.on_wait else []
                if len(waits) > max_waits:
                    rest, keep = waits[:-max_waits], waits[-max_waits:]
                    while rest:
                        chunk = rest[:max_waits]
                        rest = rest[max_waits:]
                        counter += 1
                        nop = mybir.InstNoOp(
                            name=f"WSPD-{counter}", ins=[], outs=[]
                        )
                        nop.engine = inst.engine
                        nop.sync_info = mybir.SyncInfo(
                            on_wait=chunk, on_update=[]
                        )
                        out.append(nop)
                    si.on_wait = keep
                out.append(inst)
            blk.instructions = out
    return nc


def _build():
    from contextlib import ExitStack

    import concourse.bass as bass
    import concourse.tile as tile
    from concourse import mybir
    from concourse.masks import make_identity

    F32 = mybir.dt.float32
    F32R = mybir.dt.float32r
    EXP = mybir.ActivationFunctionType.Exp

    def r(ap):
        return ap.bitcast(F32R)

    nc = bass.Bass()
    q_ext = nc.declare_dram_parameter("q", [S, E], F32, isOutput=False)
    k_ext = nc.declare_dram_parameter("k", [S, E], F32, isOutput=False)
    v_ext = nc.declare_dram_parameter("v", [S, E], F32, isOutput=False)
    wq_ext = nc.declare_dram_parameter("wq", [D, E], F32, isOutput=False)
    wk_ext = nc.declare_dram_parameter("wk", [D, E], F32, isOutput=False)
    wv_ext = nc.declare_dram_parameter("wv", [D, E], F32, isOutput=False)
    att_ext = nc.declare_dram_parameter("att", [S, D], F32, isOutput=True)
    score_ext = nc.declare_dram_parameter("score", [S, S], F32, isOutput=True)

    with tile.TileContext(nc) as tc, ExitStack() as ctx:
        singles = ctx.enter_context(tc.tile_pool(name="singles", bufs=1))
        ident = singles.tile([128, 128], F32)
        make_identity(nc, ident[:])

        persist = ctx.enter_context(tc.tile_pool(name="persist", bufs=1))
        qT = persist.tile([64, S], F32)          # Q.T  [d, s]
        kT = persist.tile([64, S], F32)          # K.T  [d, s]
        vT = persist.tile([64, S], F32)          # V.T  [d, t]
        vsb = persist.tile([128, NSC, D], F32)   # V natural [t, d] per chunk
        wqT = persist.tile([128, NEC, D], F32)   # W.T [e, d] per e-chunk
        wkT = persist.tile([128, NEC, D], F32)
        wvT = persist.tile([128, NEC, D], F32)
        sums_part = persist.tile([128, NSC, 2], F32)
        recip = persist.tile([128, NSC], F32)    # 1/rowsum per s-chunk
        attT = persist.tile([64, S], F32)        # attention.T unnormalized
        att_sb = persist.tile([128, NSC, D], F32)

        # ---- Phase 0: weights -> wT [e, d] ------------------------------
        with (
            tc.tile_pool(name="wload", bufs=2) as wload,
            tc.tile_pool(name="tps", bufs=2, space="PSUM") as tps0,
        ):
            for w_ext, wT in ((wq_ext, wqT), (wk_ext, wkT), (wv_ext, wvT)):
                wnat = wload.tile([64, E], F32)
                nc.sync.dma_start(out=wnat[:], in_=w_ext[:])
                for ec in range(NEC):
                    ps = tps0.tile([128, D], F32)
                    nc.tensor.transpose(
                        ps[:], wnat[:, ec * 128 : (ec + 1) * 128],
                        ident[:64, :64],
                    )
                    nc.vector.tensor_copy(out=r(wT[:, ec, :]), in_=ps[:])

        # ---- Phase 1: x -> x.T -> projections ---------------------------
        with (
            tc.tile_pool(name="xload", bufs=6) as xload,
            tc.tile_pool(name="xT", bufs=2) as xTp,
            tc.tile_pool(name="tpps", bufs=3, space="PSUM") as tpps,
            tc.tile_pool(name="projps", bufs=2, space="PSUM") as projps,
            tc.tile_pool(name="vtps", bufs=2, space="PSUM") as vtps,
        ):
            for x_ext, wT, dstT in (
                (q_ext, wqT, qT),
                (k_ext, wkT, kT),
                (v_ext, wvT, vT),
            ):
                for sb in range(NSB):
                    xT = xTp.tile([128, NEC, 512], F32, tag="xT")
                    for sc4 in range(4):
                        sc = sb * 4 + sc4
                        xnat = xload.tile([128, E], F32, tag="xnat")
                        nc.sync.dma_start(
                            out=xnat[:], in_=x_ext[sc * 128 : (sc + 1) * 128, :]
                        )
                        for g in range(2):
                            tp = tpps.tile([128, 512], F32, tag="tp")
                            for j in range(4):
                                ec = g * 4 + j
                                nc.tensor.transpose(
                                    tp[:, j * 128 : (j + 1) * 128],
                                    xnat[:, ec * 128 : (ec + 1) * 128],
                                    ident[:],
                                )
                            dst = xT[:, g * 4 : (g + 1) * 4,
                                     sc4 * 128 : (sc4 + 1) * 128]
                            src = tp[:].rearrange("p (a b) -> p a b", a=4)
                            if g == 0:
                                nc.scalar.copy(out=r(dst), in_=src)
                            else:
                                nc.vector.tensor_copy(out=r(dst), in_=src)
                    ps = projps.tile([64, 512], F32, tag="proj")
                    for ec in range(NEC):
                        nc.tensor.matmul(
                            ps[:],
                            r(wT[:, ec, :]),
                            r(xT[:, ec, :]),
                            start=(ec == 0),
                            stop=(ec == NEC - 1),
                        )
                    nc.scalar.copy(
                        out=r(dstT[:, sb * 512 : (sb + 1) * 512]), in_=ps[:]
                    )
            # V natural layout [t, d] from vT
            for tcn in range(NSC):
                ps = vtps.tile([128, D], F32, tag="vt")
                nc.tensor.transpose(
                    ps[:], vT[:, tcn * 128 : (tcn + 1) * 128], ident[:64, :64]
                )
                nc.vector.tensor_copy(out=r(vsb[:, tcn, :]), in_=ps[:])

        # ---- Phase 2: scores / softmax / PV, per s-block ----------------
        with (
            tc.tile_pool(name="mmps", bufs=3, space="PSUM") as mmps,
            tc.tile_pool(name="pvps", bufs=2, space="PSUM") as pvps,
            tc.tile_pool(name="expT", bufs=3) as expTp,
            tc.tile_pool(name="exps", bufs=2) as expsp,
            tc.tile_pool(name="pn", bufs=3) as pnp,
            tc.tile_pool(name="small", bufs=4) as smallp,
        ):
            for sb in range(NSB):
                sblk = slice(sb * 512, (sb + 1) * 512)
                # scoresT -> exp -> PV (unnormalized)
                pv = pvps.tile([64, 512], F32, tag="pv")
                for tc2 in range(NSC // 2):
                    sT = mmps.tile([128, 1024], F32, tag="mm")
                    for j in range(2):
                        tcn = tc2 * 2 + j
                        nc.tensor.matmul(
                            sT[:, j * 512 : (j + 1) * 512],
                            r(kT[:, tcn * 128 : (tcn + 1) * 128]),
                            r(qT[:, sblk]),
                            start=True,
                            stop=True,
                        )
                    eT = expTp.tile([128, 1024], F32, tag="eT")
                    nc.scalar.activation(
                        out=r(eT[:]), in_=sT[:], func=EXP, scale=SCALE
                    )
                    for j in range(2):
                        tcn = tc2 * 2 + j
                        nc.tensor.matmul(
                            pv[:],
                            r(vsb[:, tcn, :]),
                            r(eT[:, j * 512 : (j + 1) * 512]),
                            start=(tcn == 0),
                            stop=(tcn == NSC - 1),
                            skip_group_check=True,
                        )
                nc.scalar.copy(out=attT[:, sblk], in_=pv[:])

                # scores rows -> exp(+accum) -> normalize -> score out
                for sc4 in range(4):
                    sc = sb * 4 + sc4
                    srow = slice(sc * 128, (sc + 1) * 128)
                    exp_sb = expsp.tile([128, S], F32, tag="exp")
                    for h in range(2):
                        sps = mmps.tile([128, 1024], F32, tag="mm")
                        for n2 in range(2):
                            t0 = h * 1024 + n2 * 512
                            nc.tensor.matmul(
                                sps[:, n2 * 512 : (n2 + 1) * 512],
                                r(qT[:, srow]),
                                r(kT[:, t0 : t0 + 512]),
                                start=True,
                                stop=True,
                            )
                        nc.scalar.activation(
                            out=exp_sb[:, h * 1024 : (h + 1) * 1024],
                            in_=sps[:],
                            func=EXP,
                            scale=SCALE,
                            accum_out=sums_part[:, sc, h : h + 1],
                        )
                    stmp = smallp.tile([128, 1], F32, tag="stmp")
                    nc.vector.tensor_add(
                        stmp[:], sums_part[:, sc, 0:1], sums_part[:, sc, 1:2]
                    )
                    nc.vector.reciprocal(recip[:, sc : sc + 1], stmp[:])
                    pn = pnp.tile([128, S], F32, tag="pn")
                    nc.vector.tensor_scalar_mul(
                        pn[:], exp_sb[:], recip[:, sc : sc + 1]
                    )
                    nc.sync.dma_start(out=score_ext[srow, :], in_=pn[:])

        # ---- Phase 3: attention = attT.T * recip ------------------------
        with tc.tile_pool(name="attps", bufs=4, space="PSUM") as attps:
            for sc in range(NSC):
                ps = attps.tile([128, D], F32, tag="attp")
                nc.tensor.transpose(
                    ps[:], attT[:, sc * 128 : (sc + 1) * 128], ident[:64, :64]
                )
                nc.vector.tensor_scalar_mul(
                    att_sb[:, sc, :], ps[:], recip[:, sc : sc + 1]
                )
            nc.sync.dma_start(
                out=att_ext[:].rearrange("(c p) d -> p c d", p=128),
                in_=att_sb[:],
            )

    _spread_sync_waits(nc, mybir)
    return nc


def _get_nc():
    if "nc" not in _CACHE:
        _CACHE["nc"] = _build()
    return _CACHE["nc"]


def kernel(query, key, value, mask, WQ, WK, WV):
    from concourse.bass_utils import run_bass_kernel_spmd

    nc = _get_nc()
    query = np.ascontiguousarray(np.asarray(query, dtype=np.float32))
    key = np.ascontiguousarray(np.asarray(key, dtype=np.float32))
    value = np.ascontiguousarray(np.asarray(value, dtype=np.float32))
    WQ = np.ascontiguousarray(np.asarray(WQ, dtype=np.float32))
    WK = np.ascontiguousarray(np.asarray(WK, dtype=np.float32))
    WV = np.ascontiguousarray(np.asarray(WV, dtype=np.float32))

    in_maps = [
        {
            "q": query[b],
            "k": key[b],
            "v": value[b],
            "wq": WQ,
            "wk": WK,
            "wv": WV,
        }
        for b in range(N_CORES)
    ]
    res = run_bass_kernel_spmd(nc, in_maps, core_ids=list(range(N_CORES)))
    att = np.stack([res.results[b]["att"] for b in range(N_CORES)])
    score = np.stack([res.results[b]["score"] for b in range(N_CORES)])
    return att, score


# revision 2
# speedup vs baseline: 1.0999x; 1.0999x over previous
"""Single-head attention on 8 TRN2 NeuronCores, data-parallel over batch.

Per core (batch b):
  x (q/k/v) loaded with an f32->bf16 cast DMA (gpsimd SWDGE), transposed on
  the PE in bf16 (128x128 tiles), projected with bf16 matmuls into
  qT/kT/vT [64,2048] (fp32 PSUM accumulation, stored as f32r).
  scores[s,t] tiles -> exp(scale*x) with fused row-sum accum -> normalize
    (DVE) -> attention_score rows out.
  scoresT[t,s] tiles -> exp -> PV matmul (V natural, from vT transpose)
    -> attT -> transpose + per-row 1/sum -> attention out.
  Score-side matmuls run as float32r (full-rate fp32 at N>=512).

WQ/WK/WV are transposed + bf16-cast host-side (parameter prep only).
"""

import numpy as np

B, S, E, D = 8, 2048, 1024, 64
N_CORES = 8
NSC = S // 128   # 16 s-chunks
NEC = E // 128   # 8  e-chunks
NSB = S // 512   # 4  s-blocks
SCALE = 0.125    # 1 / sqrt(D)

MAX_SYNC_WAITS = 1

_CACHE = {}


def _spread_sync_waits(nc, mybir, max_waits=MAX_SYNC_WAITS):
    """This walrus rejects instructions with more than `max_waits` sync-waits
    ("Too many sync wait commands"). Hoist excess waits onto same-engine NoOps
    placed immediately before the instruction."""
    counter = 0
    for f in nc.m.functions:
        for blk in f.blocks:
            out = []
            for inst in blk.instructions:
                si = getattr(inst, "sync_info", None)
                waits = list(si.on_wait) if si is not None and si.on_wait else []
                if len(waits) > max_waits:
                    rest, keep = waits[:-max_waits], waits[-max_waits:]
                    while rest:
                        chunk = rest[:max_waits]
                        rest = rest[max_waits:]
                        counter += 1
                        nop = mybir.InstNoOp(
                            name=f"WSPD-{counter}", ins=[], outs=[]
                        )
                        nop.engine = inst.engine
                        nop.sync_info = mybir.SyncInfo(
                            on_wait=chunk, on_update=[]
                        )
                        out.append(nop)
                    si.on_wait = keep
                out.append(inst)
            blk.instructions = out
    return nc


def _build():
    from contextlib import ExitStack

    import concourse.bass as bass
    import concourse.tile as tile
    from concourse import mybir
    from concourse.masks import make_identity

    F32 = mybir.dt.float32
    F32R = mybir.dt.float32r
    BF16 = mybir.dt.bfloat16
    EXP = mybir.ActivationFunctionType.Exp

    def r(ap):
        return ap.bitcast(F32R)

    nc = bass.Bass()
    q_ext = nc.declare_dram_parameter("q", [S, E], F32, isOutput=False)
    k_ext = nc.declare_dram_parameter("k", [S, E], F32, isOutput=False)
    v_ext = nc.declare_dram_parameter("v", [S, E], F32, isOutput=False)
    # weights arrive transposed [E, D] and bf16-cast (host-side prep)
    wq_ext = nc.declare_dram_parameter("wqt", [E, D], BF16, isOutput=False)
    wk_ext = nc.declare_dram_parameter("wkt", [E, D], BF16, isOutput=False)
    wv_ext = nc.declare_dram_parameter("wvt", [E, D], BF16, isOutput=False)
    att_ext = nc.declare_dram_parameter("att", [S, D], F32, isOutput=True)
    score_ext = nc.declare_dram_parameter("score", [S, S], F32, isOutput=True)

    with tile.TileContext(nc) as tc, ExitStack() as ctx:
        singles = ctx.enter_context(tc.tile_pool(name="singles", bufs=1))
        ident = singles.tile([128, 128], F32)
        make_identity(nc, ident[:])
        identb = singles.tile([128, 128], BF16)
        make_identity(nc, identb[:])

        persist = ctx.enter_context(tc.tile_pool(name="persist", bufs=1))
        qT = persist.tile([64, S], F32)          # Q.T  [d, s]
        kT = persist.tile([64, S], F32)          # K.T  [d, s]
        vT = persist.tile([64, S], F32)          # V.T  [d, t]
        vsb = persist.tile([128, NSC, D], F32)   # V natural [t, d] per chunk
        wqT = persist.tile([128, NEC, D], BF16)  # W.T [e, d] per e-chunk
        wkT = persist.tile([128, NEC, D], BF16)
        wvT = persist.tile([128, NEC, D], BF16)
        xTq = persist.tile([128, NEC, S], BF16)  # x.T per e-chunk
        xTk = persist.tile([128, NEC, S], BF16)
        xTv = persist.tile([128, NEC, S], BF16)
        sums_part = persist.tile([128, NSC, 2], F32)
        recip = persist.tile([128, NSC], F32)    # 1/rowsum per s-chunk
        attT = persist.tile([64, S], F32)        # attention.T unnormalized
        att_sb = persist.tile([128, NSC, D], F32)

        # weights: direct strided DMA into [e-in-chunk, e-chunk, d]
        for w_ext, wT in ((wq_ext, wqT), (wk_ext, wkT), (wv_ext, wvT)):
            nc.sync.dma_start(
                out=wT[:], in_=w_ext[:].rearrange("(c p) d -> p c d", p=128)
            )

        # ---- Phase 1: x -> x.T (bf16) -> projections ---------------------
        with (
            tc.tile_pool(name="xload", bufs=6) as xload,
            tc.tile_pool(name="warmps", bufs=1, space="PSUM") as warmps,
            tc.tile_pool(name="tpps", bufs=3, space="PSUM") as tpps,
            tc.tile_pool(name="projps", bufs=2, space="PSUM") as projps,
            tc.tile_pool(name="vtps", bufs=2, space="PSUM") as vtps,
        ):
            # HAM warmup: dense stream of real bf16 matmuls while DMAs load
            wps = warmps.tile([128, 128], F32)
            for i in range(48):
                nc.tensor.matmul(
                    wps[:], identb[:], identb[:], start=True, stop=True
                )

            for x_ext, wT, xT, dstT in (
                (q_ext, wqT, xTq, qT),
                (k_ext, wkT, xTk, kT),
                (v_ext, wvT, xTv, vT),
            ):
                for sb in range(NSB):
                    for sc4 in range(4):
                        sc = sb * 4 + sc4
                        xnat = xload.tile([128, E], BF16, tag="xnat")
                        nc.gpsimd.dma_start(
                            out=xnat[:], in_=x_ext[sc * 128 : (sc + 1) * 128, :]
                        )
                        for g in range(2):
                            tp = tpps.tile([128, 512], BF16, tag="tp")
                            for j in range(4):
                                ec = g * 4 + j
                                nc.tensor.transpose(
                                    tp[:, j * 128 : (j + 1) * 128],
                                    xnat[:, ec * 128 : (ec + 1) * 128],
                                    identb[:],
                                )
                            dst = xT[:, g * 4 : (g + 1) * 4,
                                     sc * 128 : (sc + 1) * 128]
                            src = tp[:].rearrange("p (a b) -> p a b", a=4)
                            if g == 0:
                                nc.scalar.copy(out=dst, in_=src)
                            else:
                                nc.vector.tensor_copy(out=dst, in_=src)
                    ps = projps.tile([64, 512], F32, tag="proj")
                    sblk = slice(sb * 512, (sb + 1) * 512)
                    for ec in range(NEC):
                        nc.tensor.matmul(
                            ps[:],
                            wT[:, ec, :],
                            xT[:, ec, sblk],
                            start=(ec == 0),
                            stop=(ec == NEC - 1),
                        )
                    nc.scalar.copy(out=r(dstT[:, sblk]), in_=ps[:])
            # V natural layout [t, d] from vT
            for tcn in range(NSC):
                ps = vtps.tile([128, D], F32, tag="vt")
                nc.tensor.transpose(
                    ps[:], vT[:, tcn * 128 : (tcn + 1) * 128], ident[:64, :64]
                )
                nc.vector.tensor_copy(out=r(vsb[:, tcn, :]), in_=ps[:])

        # ---- Phase 2: scores / softmax / PV, per s-block ----------------
        with (
            tc.tile_pool(name="mmps", bufs=3, space="PSUM") as mmps,
            tc.tile_pool(name="pvps", bufs=2, space="PSUM") as pvps,
            tc.tile_pool(name="expT", bufs=3) as expTp,
            tc.tile_pool(name="exps", bufs=3) as expsp,
            tc.tile_pool(name="small", bufs=4) as smallp,
        ):
            for sb in range(NSB):
                sblk = slice(sb * 512, (sb + 1) * 512)
                # scoresT -> exp -> PV (unnormalized)
                pv = pvps.tile([64, 512], F32, tag="pv")
                for tc2 in range(NSC // 2):
                    sT = mmps.tile([128, 1024], F32, tag="mm")
                    for j in range(2):
                        tcn = tc2 * 2 + j
                        nc.tensor.matmul(
                            sT[:, j * 512 : (j + 1) * 512],
                            r(kT[:, tcn * 128 : (tcn + 1) * 128]),
                            r(qT[:, sblk]),
                            start=True,
                            stop=True,
                        )
                    eT = expTp.tile([128, 1024], F32, tag="eT")
                    nc.scalar.activation(
                        out=r(eT[:]), in_=sT[:], func=EXP, scale=SCALE
                    )
                    for j in range(2):
                        tcn = tc2 * 2 + j
                        nc.tensor.matmul(
                            pv[:],
                            r(vsb[:, tcn, :]),
                            r(eT[:, j * 512 : (j + 1) * 512]),
                            start=(tcn == 0),
                            stop=(tcn == NSC - 1),
                            skip_group_check=True,
                        )
                nc.scalar.copy(out=attT[:, sblk], in_=pv[:])

                # scores rows -> exp(+accum) -> normalize -> score out
                for sc4 in range(4):
                    sc = sb * 4 + sc4
                    srow = slice(sc * 128, (sc + 1) * 128)
                    exp_sb = expsp.tile([128, S], F32, tag="exp")
                    for h in range(2):
                        sps = mmps.tile([128, 1024], F32, tag="mm")
                        for n2 in range(2):
                            t0 = h * 1024 + n2 * 512
                            nc.tensor.matmul(
                                sps[:, n2 * 512 : (n2 + 1) * 512],
                                r(qT[:, srow]),
                                r(kT[:, t0 : t0 + 512]),
                                start=True,
                                stop=True,
                            )
                        nc.scalar.activation(
                            out=exp_sb[:, h * 1024 : (h + 1) * 1024],
                            in_=sps[:],
                            func=EXP,
                            scale=SCALE,
                            accum_out=sums_part[:, sc, h : h + 1],
                        )
                    stmp = smallp.tile([128, 1], F32, tag="stmp")
                    nc.vector.tensor_add(
                        stmp[:], sums_part[:, sc, 0:1], sums_part[:, sc, 1:2]
                    )
                    nc.vector.reciprocal(recip[:, sc : sc + 1], stmp[:])
                    nc.vector.tensor_scalar_mul(
                        exp_sb[:], exp_sb[:], recip[:, sc : sc + 1]
                    )
                    nc.sync.dma_start(out=score_ext[srow, :], in_=exp_sb[:])

        # ---- Phase 3: attention = attT.T * recip ------------------------
        with tc.tile_pool(name="attps", bufs=4, space="PSUM") as attps:
            for sc in range(NSC):
                ps = attps.tile([128, D], F32, tag="attp")
                nc.tensor.transpose(
                    ps[:], attT[:, sc * 128 : (sc + 1) * 128], ident[:64, :64]
                )
                nc.vector.tensor_scalar_mul(
                    att_sb[:, sc, :], ps[:], recip[:, sc : sc + 1]
                )
            nc.sync.dma_start(
                out=att_ext[:].rearrange("(c p) d -> p c d", p=128),
                in_=att_sb[:],
            )

    _spread_sync_waits(nc, mybir)
    return nc


def _get_nc():
    if "nc" not in _CACHE:
        _CACHE["nc"] = _build()
    return _CACHE["nc"]


def _make_in_maps(query, key, value, WQ, WK, WV):
    import ml_dtypes

    bf16 = ml_dtypes.bfloat16
    query = np.ascontiguousarray(np.asarray(query, dtype=np.float32))
    key = np.ascontiguousarray(np.asarray(key, dtype=np.float32))
    value = np.ascontiguousarray(np.asarray(value, dtype=np.float32))
    wqt = np.ascontiguousarray(np.asarray(WQ, dtype=np.float32).T.astype(bf16))
    wkt = np.ascontiguousarray(np.asarray(WK, dtype=np.float32).T.astype(bf16))
    wvt = np.ascontiguousarray(np.asarray(WV, dtype=np.float32).T.astype(bf16))
    return [
        {
            "q": query[b],
            "k": key[b],
            "v": value[b],
            "wqt": wqt,
            "wkt": wkt,
            "wvt": wvt,
        }
        for b in range(N_CORES)
    ]


def kernel(query, key, value, mask, WQ, WK, WV):
    from concourse.bass_utils import run_bass_kernel_spmd

    nc = _get_nc()
    in_maps = _make_in_maps(query, key, value, WQ, WK, WV)
    res = run_bass_kernel_spmd(nc, in_maps, core_ids=list(range(N_CORES)))
    att = np.stack([res.results[b]["att"] for b in range(N_CORES)])
    score = np.stack([res.results[b]["score"] for b in range(N_CORES)])
    return att, score


# revision 3
# speedup vs baseline: 1.1931x; 1.0847x over previous
"""Single-head attention on 8 TRN2 NeuronCores, data-parallel over batch.

Per core (batch b):
  x (q/k/v) loaded with an f32->bf16 cast DMA (gpsimd SWDGE), transposed on
  the PE via normal bf16 matmuls against an identity moving operand (FWL
  weight path, counts as PE-busy for the HAM clock gate), projected with
  bf16 matmuls (fp32 PSUM accumulation).
  Projections are column-tiled pairs producing qT/kT duplicated on both
  partition halves; scores[s,t] and scoresT[t,s] then run as row-tiled
  concurrent pairs (K=64 each, two 64-row groups of the PE at once).
  exp(scale*x) on ScalarE with fused row-sum accumulation; normalization on
  VectorE; P.V from the [t,s] orientation; attention normalized at the end.

WQ/WK/WV are transposed + bf16-cast host-side (parameter prep only).
"""

import numpy as np

B, S, E, D = 8, 2048, 1024, 64
N_CORES = 8
NSC = S // 128   # 16 s-chunks
NEC = E // 128   # 8  e-chunks
NSB = S // 512   # 4  s-blocks
SCALE = 0.125    # 1 / sqrt(D)

MAX_SYNC_WAITS = 1

_CACHE = {}


def _spread_sync_waits(nc, mybir, max_waits=MAX_SYNC_WAITS):
    """This walrus rejects instructions with more than `max_waits` sync-waits
    ("Too many sync wait commands"). Hoist excess waits onto same-engine NoOps
    placed immediately before the instruction."""
    counter = 0
    for f in nc.m.functions:
        for blk in f.blocks:
            out = []
            for inst in blk.instructions:
                si = getattr(inst, "sync_info", None)
                waits = list(si.on_wait) if si is not None and si.on_wait else []
                if len(waits) > max_waits:
                    rest, keep = waits[:-max_waits], waits[-max_waits:]
                    while rest:
                        chunk = rest[:max_waits]
                        rest = rest[max_waits:]
                        counter += 1
                        nop = mybir.InstNoOp(
                            name=f"WSPD-{counter}", ins=[], outs=[]
                        )
                        nop.engine = inst.engine
                        nop.sync_info = mybir.SyncInfo(
                            on_wait=chunk, on_update=[]
                        )
                        out.append(nop)
                    si.on_wait = keep
                out.append(inst)
            blk.instructions = out
    return nc


def _build():
    from contextlib import ExitStack

    import concourse.bass as bass
    import concourse.tile as tile
    from concourse import mybir
    from concourse.masks import make_identity

    F32 = mybir.dt.float32
    BF16 = mybir.dt.bfloat16
    EXP = mybir.ActivationFunctionType.Exp

    nc = bass.Bass()
    q_ext = nc.declare_dram_parameter("q", [S, E], F32, isOutput=False)
    k_ext = nc.declare_dram_parameter("k", [S, E], F32, isOutput=False)
    v_ext = nc.declare_dram_parameter("v", [S, E], F32, isOutput=False)
    # weights arrive transposed [E, D] and bf16-cast (host-side prep)
    wq_ext = nc.declare_dram_parameter("wqt", [E, D], BF16, isOutput=False)
    wk_ext = nc.declare_dram_parameter("wkt", [E, D], BF16, isOutput=False)
    wv_ext = nc.declare_dram_parameter("wvt", [E, D], BF16, isOutput=False)
    att_ext = nc.declare_dram_parameter("att", [S, D], F32, isOutput=True)
    score_ext = nc.declare_dram_parameter("score", [S, S], F32, isOutput=True)

    with tile.TileContext(nc) as tc, ExitStack() as ctx:
        singles = ctx.enter_context(tc.tile_pool(name="singles", bufs=1))
        ident = singles.tile([128, 128], F32)
        make_identity(nc, ident[:])
        identb = singles.tile([128, 128], BF16)
        make_identity(nc, identb[:])

        persist = ctx.enter_context(tc.tile_pool(name="persist", bufs=1))
        qT2 = persist.tile([128, S], BF16)       # Q.T dup on both halves
        kT2 = persist.tile([128, S], BF16)       # K.T dup on both halves
        vT = persist.tile([64, S], BF16)         # V.T  [d, t]
        vsb = persist.tile([128, NSC, D], BF16)  # V natural [t, d] per chunk
        wqT = persist.tile([128, NEC, D], BF16)  # W.T [e, d] per e-chunk
        wkT = persist.tile([128, NEC, D], BF16)
        wvT = persist.tile([128, NEC, D], BF16)
        xTq = persist.tile([128, NEC, S], BF16)  # x.T per e-chunk
        xTk = persist.tile([128, NEC, S], BF16)
        xTv = persist.tile([128, NEC, S], BF16)
        sums_part = persist.tile([128, NSC, 2], F32)
        recip = persist.tile([128, NSC], F32)    # 1/rowsum per s-chunk
        attT = persist.tile([64, S], F32)        # attention.T unnormalized
        att_sb = persist.tile([128, NSC, D], F32)

        # weights: direct strided DMA into [e-in-chunk, e-chunk, d]
        for w_ext, wT in ((wq_ext, wqT), (wk_ext, wkT), (wv_ext, wvT)):
            nc.sync.dma_start(
                out=wT[:], in_=w_ext[:].rearrange("(c p) d -> p c d", p=128)
            )

        # ---- Phase 1: x -> x.T (bf16) -> projections ---------------------
        with (
            tc.tile_pool(name="xload", bufs=6) as xload,
            tc.tile_pool(name="warmps", bufs=1, space="PSUM") as warmps,
            tc.tile_pool(name="tpps", bufs=3, space="PSUM") as tpps,
            tc.tile_pool(name="projps", bufs=2, space="PSUM") as projps,
        ):
            # HAM warmup: dense stream of real bf16 matmuls while DMAs load
            wps = warmps.tile([128, 128], F32)
            for i in range(48):
                nc.tensor.matmul(
                    wps[:], identb[:], identb[:], start=True, stop=True
                )

            copy_flip = [0]

            def copy_alt(out, in_):
                if copy_flip[0] % 2 == 0:
                    nc.scalar.copy(out=out, in_=in_)
                else:
                    nc.vector.tensor_copy(out=out, in_=in_)
                copy_flip[0] += 1

            for x_ext, wT, xT, dstT, dup in (
                (q_ext, wqT, xTq, qT2, True),
                (k_ext, wkT, xTk, kT2, True),
                (v_ext, wvT, xTv, vT, False),
            ):
                for sb in range(NSB):
                    for sc4 in range(4):
                        sc = sb * 4 + sc4
                        xnat = xload.tile([128, E], BF16, tag="xnat")
                        nc.gpsimd.dma_start(
                            out=xnat[:], in_=x_ext[sc * 128 : (sc + 1) * 128, :]
                        )
                        for g in range(2):
                            tp = tpps.tile([128, 512], F32, tag="tp")
                            for j in range(4):
                                ec = g * 4 + j
                                # out = xnat_chunk.T @ I  (normal bf16 matmul)
                                nc.tensor.matmul(
                                    tp[:, j * 128 : (j + 1) * 128],
                                    xnat[:, ec * 128 : (ec + 1) * 128],
                                    identb[:],
                                    start=True,
                                    stop=True,
                                )
                            dst = xT[:, g * 4 : (g + 1) * 4,
                                     sc * 128 : (sc + 1) * 128]
                            src = tp[:].rearrange("p (a b) -> p a b", a=4)
                            copy_alt(dst, src)
                    sblk = slice(sb * 512, (sb + 1) * 512)
                    if dup:
                        # column-tiled pair -> result duplicated on both
                        # partition halves of one [128,512] psum tile
                        ps = projps.tile([128, 512], F32, tag="proj")
                        for ec in range(NEC):
                            nc.tensor.matmul(
                                ps[0:64, :],
                                wT[:, ec, :],
                                xT[:, ec, sblk],
                                start=(ec == 0),
                                stop=(ec == NEC - 1),
                                tile_position=(0, 0),
                                skip_group_check=True,
                            )
                            nc.tensor.matmul(
                                ps[64:128, :],
                                wT[:, ec, :],
                                xT[:, ec, sblk],
                                start=(ec == 0),
                                stop=(ec == NEC - 1),
                                tile_position=(0, 64),
                                skip_group_check=True,
                            )
                        copy_alt(dstT[:, sblk], ps[:])
                    else:
                        ps = projps.tile([128, 512], F32, tag="proj")
                        for ec in range(NEC):
                            nc.tensor.matmul(
                                ps[0:64, :],
                                wT[:, ec, :],
                                xT[:, ec, sblk],
                                start=(ec == 0),
                                stop=(ec == NEC - 1),
                            )
                        copy_alt(dstT[:, sblk], ps[0:64, :])
            # V natural layout [t, d] from vT (normal bf16 mm vs identity)
            for tcn in range(NSC):
                ps = tpps.tile([128, 512], F32, tag="tp")
                nc.tensor.matmul(
                    ps[:, 0:64],
                    vT[:, tcn * 128 : (tcn + 1) * 128],
                    identb[0:64, 0:64],
                    start=True,
                    stop=True,
                )
                nc.vector.tensor_copy(out=vsb[:, tcn, :], in_=ps[:, 0:64])

        # ---- Phase 2: scores / softmax / PV, per s-block ----------------
        with (
            tc.tile_pool(name="mmps", bufs=3, space="PSUM") as mmps,
            tc.tile_pool(name="pvps", bufs=2, space="PSUM") as pvps,
            tc.tile_pool(name="expT", bufs=3) as expTp,
            tc.tile_pool(name="exps", bufs=3) as expsp,
            tc.tile_pool(name="small", bufs=4) as smallp,
        ):
            for sb in range(NSB):
                sblk = slice(sb * 512, (sb + 1) * 512)
                # scoresT (row-tiled pairs) -> exp -> PV (unnormalized)
                pv = pvps.tile([64, 512], F32, tag="pv")
                for tc2 in range(NSC // 2):
                    sT = mmps.tile([128, 1024], F32, tag="mm")
                    for j in range(2):
                        tcn = tc2 * 2 + j
                        lo, hi = 64 * j, 64 * (j + 1)
                        nc.tensor.matmul(
                            sT[:, j * 512 : (j + 1) * 512],
                            kT2[lo:hi, tcn * 128 : (tcn + 1) * 128],
                            qT2[lo:hi, sblk],
                            start=True,
                            stop=True,
                            tile_position=(64 * j, 0),
                            skip_group_check=True,
                        )
                    eT = expTp.tile([128, 1024], BF16, tag="eT")
                    nc.scalar.activation(
                        out=eT[:], in_=sT[:], func=EXP, scale=SCALE
                    )
                    for j in range(2):
                        tcn = tc2 * 2 + j
                        nc.tensor.matmul(
                            pv[:],
                            vsb[:, tcn, :],
                            eT[:, j * 512 : (j + 1) * 512],
                            start=(tcn == 0),
                            stop=(tcn == NSC - 1),
                            skip_group_check=True,
                        )
                nc.scalar.copy(out=attT[:, sblk], in_=pv[:])

                # scores rows (row-tiled pairs over s-chunks)
                for sp2 in range(2):  # pairs of s-chunks within the block
                    scA = sb * 4 + sp2 * 2
                    scB = scA + 1
                    rowA = slice(scA * 128, (scA + 1) * 128)
                    rowB = slice(scB * 128, (scB + 1) * 128)
                    expA = expsp.tile([128, S], F32, tag="exp")
                    expB = expsp.tile([128, S], F32, tag="exp")
                    for h in range(2):
                        th = slice(h * 1024, (h + 1) * 1024)
                        psA = mmps.tile([128, 1024], F32, tag="mm")
                        psB = mmps.tile([128, 1024], F32, tag="mm")
                        for n2 in range(2):
                            t0 = h * 1024 + n2 * 512
                            tsl = slice(t0, t0 + 512)
                            nsl = slice(n2 * 512, (n2 + 1) * 512)
                            nc.tensor.matmul(
                                psA[:, nsl],
                                qT2[0:64, rowA],
                                kT2[0:64, tsl],
                                start=True,
                                stop=True,
                                tile_position=(0, 0),
                                skip_group_check=True,
                            )
                            nc.tensor.matmul(
                                psB[:, nsl],
                                qT2[64:128, rowB],
                                kT2[64:128, tsl],
                                start=True,
                                stop=True,
                                tile_position=(64, 0),
                                skip_group_check=True,
                            )
                        nc.scalar.activation(
                            out=expA[:, th], in_=psA[:], func=EXP,
                            scale=SCALE,
                            accum_out=sums_part[:, scA, h : h + 1],
                        )
                        nc.scalar.activation(
                            out=expB[:, th], in_=psB[:], func=EXP,
                            scale=SCALE,
                            accum_out=sums_part[:, scB, h : h + 1],
                        )
                    for sc, exp_sb, srow in (
                        (scA, expA, rowA), (scB, expB, rowB),
                    ):
                        stmp = smallp.tile([128, 1], F32, tag="stmp")
                        nc.vector.tensor_add(
                            stmp[:], sums_part[:, sc, 0:1],
                            sums_part[:, sc, 1:2],
                        )
                        nc.vector.reciprocal(recip[:, sc : sc + 1], stmp[:])
                        nc.vector.tensor_scalar_mul(
                            exp_sb[:], exp_sb[:], recip[:, sc : sc + 1]
                        )
                        nc.sync.dma_start(
                            out=score_ext[srow, :], in_=exp_sb[:]
                        )

        # ---- Phase 3: attention = attT.T * recip ------------------------
        with tc.tile_pool(name="attps", bufs=4, space="PSUM") as attps:
            for sc in range(NSC):
                ps = attps.tile([128, D], F32, tag="attp")
                nc.tensor.transpose(
                    ps[:], attT[:, sc * 128 : (sc + 1) * 128], ident[:64, :64]
                )
                nc.vector.tensor_scalar_mul(
                    att_sb[:, sc, :], ps[:], recip[:, sc : sc + 1]
                )
            nc.sync.dma_start(
                out=att_ext[:].rearrange("(c p) d -> p c d", p=128),
                in_=att_sb[:],
            )

    _spread_sync_waits(nc, mybir)
    return nc


def _get_nc():
    if "nc" not in _CACHE:
        _CACHE["nc"] = _build()
    return _CACHE["nc"]


def _make_in_maps(query, key, value, WQ, WK, WV):
    import ml_dtypes

    bf16 = ml_dtypes.bfloat16
    query = np.ascontiguousarray(np.asarray(query, dtype=np.float32))
    key = np.ascontiguousarray(np.asarray(key, dtype=np.float32))
    value = np.ascontiguousarray(np.asarray(value, dtype=np.float32))
    wqt = np.ascontiguousarray(np.asarray(WQ, dtype=np.float32).T.astype(bf16))
    wkt = np.ascontiguousarray(np.asarray(WK, dtype=np.float32).T.astype(bf16))
    wvt = np.ascontiguousarray(np.asarray(WV, dtype=np.float32).T.astype(bf16))
    return [
        {
            "q": query[b],
            "k": key[b],
            "v": value[b],
            "wqt": wqt,
            "wkt": wkt,
            "wvt": wvt,
        }
        for b in range(N_CORES)
    ]


def kernel(query, key, value, mask, WQ, WK, WV):
    from concourse.bass_utils import run_bass_kernel_spmd

    nc = _get_nc()
    in_maps = _make_in_maps(query, key, value, WQ, WK, WV)
    res = run_bass_kernel_spmd(nc, in_maps, core_ids=list(range(N_CORES)))
    att = np.stack([res.results[b]["att"] for b in range(N_CORES)])
    score = np.stack([res.results[b]["score"] for b in range(N_CORES)])
    return att, score


# revision 5
# speedup vs baseline: 1.1968x; 1.0031x over previous
"""Single-head attention on 8 TRN2 NeuronCores, data-parallel over batch.

Per core (batch b):
  x (q/k/v) loaded with an f32->bf16 cast DMA (gpsimd SWDGE, 4 queues),
  transposed on the PE via normal bf16 matmuls against an identity moving
  operand, projected with bf16 matmuls (fp32 PSUM accumulation).
  Projections are column-tiled pairs producing qT/kT duplicated on both
  partition halves; scores[s,t] and scoresT[t,s] then run as row-tiled
  concurrent pairs (K=64 each, two 64-row groups of the PE at once).
  exp(scale*x) on ScalarE with fused row-sum accumulation; normalization on
  VectorE; P.V from the [t,s] orientation; attention normalized at the end.
  One shared PSUM pool lets phase-2 score work overlap the phase-1 loads.

WQ/WK/WV are transposed + bf16-cast host-side (parameter prep only).
"""

import numpy as np

B, S, E, D = 8, 2048, 1024, 64
N_CORES = 8
NSC = S // 128   # 16 s-chunks
NEC = E // 128   # 8  e-chunks
NSB = S // 512   # 4  s-blocks
SCALE = 0.125    # 1 / sqrt(D)

MAX_SYNC_WAITS = 1

_CACHE = {}


def _spread_sync_waits(nc, mybir, max_waits=MAX_SYNC_WAITS):
    """This walrus rejects instructions with more than `max_waits` sync-waits
    ("Too many sync wait commands"). Hoist excess waits onto same-engine NoOps
    placed immediately before the instruction."""
    counter = 0
    for f in nc.m.functions:
        for blk in f.blocks:
            out = []
            for inst in blk.instructions:
                si = getattr(inst, "sync_info", None)
                waits = list(si.on_wait) if si is not None and si.on_wait else []
                if len(waits) > max_waits:
                    rest, keep = waits[:-max_waits], waits[-max_waits:]
                    while rest:
                        chunk = rest[:max_waits]
                        rest = rest[max_waits:]
                        counter += 1
                        nop = mybir.InstNoOp(
                            name=f"WSPD-{counter}", ins=[], outs=[]
                        )
                        nop.engine = inst.engine
                        nop.sync_info = mybir.SyncInfo(
                            on_wait=chunk, on_update=[]
                        )
                        out.append(nop)
                    si.on_wait = keep
                out.append(inst)
            blk.instructions = out
    return nc


def _build():
    from contextlib import ExitStack

    import concourse.bass as bass
    import concourse.tile as tile
    from concourse import mybir
    from concourse.masks import make_identity

    F32 = mybir.dt.float32
    BF16 = mybir.dt.bfloat16
    EXP = mybir.ActivationFunctionType.Exp

    nc = bass.Bass(num_swdge_queues=4)
    q_ext = nc.declare_dram_parameter("q", [S, E], F32, isOutput=False)
    k_ext = nc.declare_dram_parameter("k", [S, E], F32, isOutput=False)
    v_ext = nc.declare_dram_parameter("v", [S, E], F32, isOutput=False)
    # weights arrive transposed [E, D] and bf16-cast (host-side prep)
    wq_ext = nc.declare_dram_parameter("wqt", [E, D], BF16, isOutput=False)
    wk_ext = nc.declare_dram_parameter("wkt", [E, D], BF16, isOutput=False)
    wv_ext = nc.declare_dram_parameter("wvt", [E, D], BF16, isOutput=False)
    att_ext = nc.declare_dram_parameter("att", [S, D], F32, isOutput=True)
    score_ext = nc.declare_dram_parameter("score", [S, S], F32, isOutput=True)

    with tile.TileContext(nc) as tc, ExitStack() as ctx:
        singles = ctx.enter_context(tc.tile_pool(name="singles", bufs=1))
        ident = singles.tile([128, 128], F32)
        make_identity(nc, ident[:])
        identb = singles.tile([128, 128], BF16)
        make_identity(nc, identb[:])

        persist = ctx.enter_context(tc.tile_pool(name="persist", bufs=1))
        qT2 = persist.tile([128, S], BF16)       # Q.T dup on both halves
        kT2 = persist.tile([128, S], BF16)       # K.T dup on both halves
        vT = persist.tile([64, S], BF16)         # V.T  [d, t]
        vsb = persist.tile([128, NSC, D], BF16)  # V natural [t, d] per chunk
        wqT = persist.tile([128, NEC, D], BF16)  # W.T [e, d] per e-chunk
        wkT = persist.tile([128, NEC, D], BF16)
        wvT = persist.tile([128, NEC, D], BF16)
        xTq = persist.tile([128, NEC, S], BF16)  # x.T per e-chunk
        xTk = persist.tile([128, NEC, S], BF16)
        xTv = persist.tile([128, NEC, S], BF16)
        sums_part = persist.tile([128, NSC, 2], F32)
        recip = persist.tile([128, NSC], F32)    # 1/rowsum per s-chunk
        attT = persist.tile([64, S], F32)        # attention.T unnormalized
        att_sb = persist.tile([128, NSC, D], F32)

        xload = ctx.enter_context(tc.tile_pool(name="xload", bufs=6))
        expTp = ctx.enter_context(tc.tile_pool(name="expT", bufs=3))
        expsp = ctx.enter_context(tc.tile_pool(name="exps", bufs=3))
        smallp = ctx.enter_context(tc.tile_pool(name="small", bufs=4))
        # one PSUM pool for all phases: "mm" 3x[128,1024] + "acc" 2x[128,512]
        psp = ctx.enter_context(tc.tile_pool(name="ps", bufs=1, space="PSUM"))

        def mmtile():
            return psp.tile([128, 1024], F32, tag="mm", bufs=3, name="mmt")

        def acctile():
            return psp.tile([128, 512], F32, tag="acc", bufs=2, name="acct")

        # weights: direct strided DMA into [e-in-chunk, e-chunk, d]
        for w_ext, wT in ((wq_ext, wqT), (wk_ext, wkT), (wv_ext, wvT)):
            nc.sync.dma_start(
                out=wT[:], in_=w_ext[:].rearrange("(c p) d -> p c d", p=128)
            )

        # HAM warmup: dense stream of real bf16 matmuls while DMAs load
        wps = acctile()
        for i in range(48):
            nc.tensor.matmul(
                wps[:, 0:128], identb[:], identb[:], start=True, stop=True
            )

        copy_flip = [0]

        def copy_alt(out, in_):
            if copy_flip[0] % 2 == 0:
                nc.scalar.copy(out=out, in_=in_)
            else:
                nc.vector.tensor_copy(out=out, in_=in_)
            copy_flip[0] += 1

        # ---- Phase 1: x -> x.T (bf16) -> projections ---------------------
        for x_ext, wT, xT, dstT, dup in (
            (q_ext, wqT, xTq, qT2, True),
            (k_ext, wkT, xTk, kT2, True),
            (v_ext, wvT, xTv, vT, False),
        ):
            for sb in range(NSB):
                for sc4 in range(4):
                    sc = sb * 4 + sc4
                    xnat = xload.tile([128, E], BF16, tag="xnat")
                    nc.gpsimd.dma_start(
                        out=xnat[:], in_=x_ext[sc * 128 : (sc + 1) * 128, :]
                    )
                    tp = mmtile()
                    for ec in range(NEC):
                        # out = xnat_chunk.T @ I  (normal bf16 matmul)
                        nc.tensor.matmul(
                            tp[:, ec * 128 : (ec + 1) * 128],
                            xnat[:, ec * 128 : (ec + 1) * 128],
                            identb[:],
                            start=True,
                            stop=True,
                        )
                    dst = xT[:, :, sc * 128 : (sc + 1) * 128]
                    src = tp[:].rearrange("p (a b) -> p a b", a=NEC)
                    copy_alt(dst, src)
                sblk = slice(sb * 512, (sb + 1) * 512)
                ps = acctile()
                if dup:
                    # column-tiled pair -> result duplicated on both halves
                    for ec in range(NEC):
                        nc.tensor.matmul(
                            ps[0:64, :],
                            wT[:, ec, :],
                            xT[:, ec, sblk],
                            start=(ec == 0),
                            stop=(ec == NEC - 1),
                            tile_position=(0, 0),
                            skip_group_check=True,
                        )
                        nc.tensor.matmul(
                            ps[64:128, :],
                            wT[:, ec, :],
                            xT[:, ec, sblk],
                            start=(ec == 0),
                            stop=(ec == NEC - 1),
                            tile_position=(0, 64),
                            skip_group_check=True,
                        )
                    copy_alt(dstT[:, sblk], ps[:])
                else:
                    for ec in range(NEC):
                        nc.tensor.matmul(
                            ps[0:64, :],
                            wT[:, ec, :],
                            xT[:, ec, sblk],
                            start=(ec == 0),
                            stop=(ec == NEC - 1),
                        )
                    copy_alt(dstT[:, sblk], ps[0:64, :])
        # V natural layout [t, d] from vT (normal bf16 mm vs identity)
        for tc4 in range(NSC // 4):
            ps = acctile()
            for j in range(4):
                tcn = tc4 * 4 + j
                nc.tensor.matmul(
                    ps[:, j * 128 : j * 128 + 64],
                    vT[:, tcn * 128 : (tcn + 1) * 128],
                    identb[0:64, 0:64],
                    start=True,
                    stop=True,
                )
            nc.vector.tensor_copy(
                out=vsb[:, tc4 * 4 : (tc4 + 1) * 4, :],
                in_=ps[:].rearrange("p (a b) -> p a b", a=4)[:, :, 0:64],
            )

        # ---- Phase 2: scores / softmax / PV, per s-block ----------------
        for sb in range(NSB):
            sblk = slice(sb * 512, (sb + 1) * 512)
            # scoresT (row-tiled pairs) -> exp -> PV (unnormalized)
            pv = acctile()
            for tc2 in range(NSC // 2):
                sT = mmtile()
                for j in range(2):
                    tcn = tc2 * 2 + j
                    lo, hi = 64 * j, 64 * (j + 1)
                    nc.tensor.matmul(
                        sT[:, j * 512 : (j + 1) * 512],
                        kT2[lo:hi, tcn * 128 : (tcn + 1) * 128],
                        qT2[lo:hi, sblk],
                        start=True,
                        stop=True,
                        tile_position=(64 * j, 0),
                        skip_group_check=True,
                    )
                eT = expTp.tile([128, 1024], BF16, tag="eT")
                nc.scalar.activation(
                    out=eT[:], in_=sT[:], func=EXP, scale=SCALE
                )
                for j in range(2):
                    tcn = tc2 * 2 + j
                    nc.tensor.matmul(
                        pv[0:64, :],
                        vsb[:, tcn, :],
                        eT[:, j * 512 : (j + 1) * 512],
                        start=(tcn == 0),
                        stop=(tcn == NSC - 1),
                        skip_group_check=True,
                    )
            nc.vector.tensor_copy(out=attT[:, sblk], in_=pv[0:64, :])

            # scores rows (row-tiled pairs over s-chunks)
            for sp2 in range(2):  # pairs of s-chunks within the block
                scA = sb * 4 + sp2 * 2
                scB = scA + 1
                rowA = slice(scA * 128, (scA + 1) * 128)
                rowB = slice(scB * 128, (scB + 1) * 128)
                expA = expsp.tile([128, S], F32, tag="exp")
                expB = expsp.tile([128, S], F32, tag="exp")
                for h in range(2):
                    th = slice(h * 1024, (h + 1) * 1024)
                    psA = mmtile()
                    psB = mmtile()
                    for n2 in range(2):
                        t0 = h * 1024 + n2 * 512
                        tsl = slice(t0, t0 + 512)
                        nsl = slice(n2 * 512, (n2 + 1) * 512)
                        nc.tensor.matmul(
                            psA[:, nsl],
                            qT2[0:64, rowA],
                            kT2[0:64, tsl],
                            start=True,
                            stop=True,
                            tile_position=(0, 0),
                            skip_group_check=True,
                        )
                        nc.tensor.matmul(
                            psB[:, nsl],
                            qT2[64:128, rowB],
                            kT2[64:128, tsl],
                            start=True,
                            stop=True,
                            tile_position=(64, 0),
                            skip_group_check=True,
                        )
                    nc.scalar.activation(
                        out=expA[:, th], in_=psA[:], func=EXP,
                        scale=SCALE,
                        accum_out=sums_part[:, scA, h : h + 1],
                    )
                    nc.scalar.activation(
                        out=expB[:, th], in_=psB[:], func=EXP,
                        scale=SCALE,
                        accum_out=sums_part[:, scB, h : h + 1],
                    )
                for sc, exp_sb, srow in (
                    (scA, expA, rowA), (scB, expB, rowB),
                ):
                    stmp = smallp.tile([128, 1], F32, tag="stmp")
                    nc.vector.tensor_add(
                        stmp[:], sums_part[:, sc, 0:1],
                        sums_part[:, sc, 1:2],
                    )
                    nc.vector.reciprocal(recip[:, sc : sc + 1], stmp[:])
                    nc.vector.tensor_scalar_mul(
                        exp_sb[:], exp_sb[:], recip[:, sc : sc + 1]
                    )
                    nc.sync.dma_start(
                        out=score_ext[srow, :], in_=exp_sb[:]
                    )

        # ---- Phase 3: attention = attT.T * recip ------------------------
        for sc in range(NSC):
            ps = acctile()
            nc.tensor.transpose(
                ps[:, 0:64], attT[:, sc * 128 : (sc + 1) * 128],
                ident[:64, :64],
            )
            nc.vector.tensor_scalar_mul(
                att_sb[:, sc, :], ps[:, 0:64], recip[:, sc : sc + 1]
            )
        nc.sync.dma_start(
            out=att_ext[:].rearrange("(c p) d -> p c d", p=128),
            in_=att_sb[:],
        )

    _spread_sync_waits(nc, mybir)
    return nc


def _get_nc():
    if "nc" not in _CACHE:
        _CACHE["nc"] = _build()
    return _CACHE["nc"]


def _make_in_maps(query, key, value, WQ, WK, WV):
    import ml_dtypes

    bf16 = ml_dtypes.bfloat16
    query = np.ascontiguousarray(np.asarray(query, dtype=np.float32))
    key = np.ascontiguousarray(np.asarray(key, dtype=np.float32))
    value = np.ascontiguousarray(np.asarray(value, dtype=np.float32))
    wqt = np.ascontiguousarray(np.asarray(WQ, dtype=np.float32).T.astype(bf16))
    wkt = np.ascontiguousarray(np.asarray(WK, dtype=np.float32).T.astype(bf16))
    wvt = np.ascontiguousarray(np.asarray(WV, dtype=np.float32).T.astype(bf16))
    return [
        {
            "q": query[b],
            "k": key[b],
            "v": value[b],
            "wqt": wqt,
            "wkt": wkt,
            "wvt": wvt,
        }
        for b in range(N_CORES)
    ]


def kernel(query, key, value, mask, WQ, WK, WV):
    from concourse.bass_utils import run_bass_kernel_spmd

    nc = _get_nc()
    in_maps = _make_in_maps(query, key, value, WQ, WK, WV)
    res = run_bass_kernel_spmd(nc, in_maps, core_ids=list(range(N_CORES)))
    att = np.stack([res.results[b]["att"] for b in range(N_CORES)])
    score = np.stack([res.results[b]["score"] for b in range(N_CORES)])
    return att, score
